# revision 1
# baseline (speedup 1.0000x reference)
"""DifferentialAttention on 8 TRN2 NeuronCores.

Sharding: tensor-parallel over heads (2 heads per core), no device
collectives. Each core computes qkv for its heads, causal differential
attention + per-head LayerNorm, and a partial output projection through
its slice of W_o columns; the host sums the 8 partial outputs.

All matmuls run as float32r (fp22 mantissa, full PE rate at N>=256).
"""

import numpy as np

HEAD_DIM = 64
N_HEADS = 16
D_MODEL = 2048
SEQ = 2048
LAYER_IDX = 12
LN_EPS = 1e-5
N_CORES = 8
HPC = N_HEADS // N_CORES          # heads per core = 2
CHUNK = 512                       # sq chunk width
NCHUNK = SEQ // CHUNK             # 4
NDT = D_MODEL // 128              # 16 d-tiles
NST = SEQ // 128                  # 16 s-tiles

_SYNC_CNT = [0]


def _patch_tile_drain(tile_mod, bass_rust):
    """The walrus build in this container encodes at most one sem wait per
    instruction; TileContext's exit drain carries one wait per producer
    proc. Split the extras onto single-wait NOPs."""
    from concourse.vector_clock import ScopedClock

    def patched(self, tick_clock, wait_clock):
        nc = self.nc
        drain_inst = nc.sync.drain()
        wait_clock.add_sem_waits(
            drain_inst.ins, ScopedClock({None: tick_clock.global_clock})
        )
        si = drain_inst.ins.sync_info
        waits = list(si.on_wait or [])
        if len(waits) > 1:
            si.on_wait = [waits[0]]
            for w in waits[1:]:
                nop = nc.sync.nop()
                nop.ins.sync_info = bass_rust.SyncInfo(on_wait=[w], on_update=[])
        nc.all_engine_barrier()
        popped = nc._tile_sem_poison_stack.pop()
        assert popped is self._sem_poison
        nc.clear_and_free_semaphores(list(self.sems.allocated().values()))
        nc.all_engine_barrier()

    tile_mod.TileContext._drain_and_barrier = patched


def _fix_sync_limits(nc, mybir, bass_rust):
    """Split multi-wait / multi-update instructions into single-wait NOP
    chains on the same engine queue (walrus single-sync-slot limit)."""

    def nop(engine, wait=None, update=None):
        _SYNC_CNT[0] += 1
        n = mybir.InstNoOp(name=f"syncsplit-{_SYNC_CNT[0]}", ins=[], outs=[])
        n.engine = engine
        n.sync_info = bass_rust.SyncInfo(
            on_wait=[wait] if wait is not None else [],
            on_update=[update] if update is not None else [],
        )
        return n

    for f in nc.m.functions:
        for b in f.blocks:
            out = []
            for inst in b.instructions:
                si = inst.sync_info
                post = []
                if si is not None:
                    waits = list(si.on_wait or [])
                    if len(waits) > 1:
                        for w in waits[:-1]:
                            out.append(nop(inst.engine, wait=w))
                        si.on_wait = [waits[-1]]
                    ups = list(si.on_update or [])
                    if len(ups) > 1:
                        si.on_update = [ups[0]]
                        for u in ups[1:]:
                            post.append(nop(inst.engine, update=u))
                out.append(inst)
                out.extend(post)
            b.instructions = out


def _install_ntff_shim():
    """Register the axon NTFF profile hook (used only when tracing)."""
    import sys, types
    if "antenv.axon_hooks" in sys.modules:
        return
    try:
        mod = types.ModuleType("antenv.axon_hooks")
        mod._hook = None
        mod.set_axon_ntff_profile_hook = lambda h: setattr(mod, "_hook", h)
        mod.get_axon_ntff_profile_hook = lambda: mod._hook
        sys.modules["antenv.axon_hooks"] = mod
        import antenv
        antenv.axon_hooks = mod
        from trn_agent_boot.trn_boot import _ntff_profile_via_ctypes
        mod.set_axon_ntff_profile_hook(
            _ntff_profile_via_ctypes("/opt/axon/libaxon_pjrt.so")
        )
    except Exception:
        pass


def _build_nc():
    import bass_rust
    import concourse.bass as bass
    import concourse.tile as tile
    from concourse import mybir

    _patch_tile_drain(tile, bass_rust)

    f32 = mybir.dt.float32
    f32r = mybir.dt.float32r
    bf16 = mybir.dt.bfloat16
    AT = mybir.ActivationFunctionType
    OP = mybir.AluOpType

    nc = bass.Bass()

    xT = nc.dram_tensor("xT", [D_MODEL, SEQ], bf16, kind="ExternalInput")
    wqkT = nc.dram_tensor("wqkT", [D_MODEL, 4 * 128], bf16, kind="ExternalInput")
    wvT = nc.dram_tensor("wvT", [D_MODEL, HPC * 128], bf16, kind="ExternalInput")
    woT = nc.dram_tensor("woT", [HPC * 128, D_MODEL], bf16, kind="ExternalInput")
    lamneg = nc.dram_tensor("lamneg", [128], f32, kind="ExternalInput")
    gamma = nc.dram_tensor("gamma", [HPC, 128], f32, kind="ExternalInput")
    beta = nc.dram_tensor("beta", [HPC, 128], f32, kind="ExternalInput")
    trimask = nc.dram_tensor("trimask", [128, 128], f32r, kind="ExternalInput")
    onesin = nc.dram_tensor("onesin", [128, 128], f32r, kind="ExternalInput")
    meanin = nc.dram_tensor("meanin", [128, 1], f32r, kind="ExternalInput")
    y = nc.dram_tensor("y", [SEQ, D_MODEL], bf16, kind="ExternalOutput")

    with tile.TileContext(nc) as tc, nc.allow_low_precision(reason="fp32r pipeline"):
        import contextlib
        with contextlib.ExitStack() as ctx:
            consts = ctx.enter_context(tc.tile_pool(name="consts", bufs=1))
            main = ctx.enter_context(tc.tile_pool(name="main", bufs=1))
            drp = ctx.enter_context(tc.tile_pool(name="drp", bufs=4, space="DRAM"))

            # ---- constants (loads deferred behind chunk-0 data) ----
            lam_sb = consts.tile([128, 1], f32)
            gam_sb = [consts.tile([128, 1], f32, name=f"gam{h}") for h in range(HPC)]
            bet_sb = [consts.tile([128, 1], f32, name=f"bet{h}") for h in range(HPC)]
            tri_sb = consts.tile([128, 128], f32r)
            ones_col = consts.tile([128, 1], f32r)
            mean_col = consts.tile([128, 1], f32r)

            def load_consts():
                nc.sync.dma_start(lam_sb[:, 0], lamneg[:])
                for h in range(HPC):
                    nc.sync.dma_start(gam_sb[h][:, 0], gamma[h, :])
                    nc.sync.dma_start(bet_sb[h][:, 0], beta[h, :])
                nc.sync.dma_start(tri_sb[:], trimask[:])
                nc.sync.dma_start(ones_col[:], onesin[:, 0:1])
                nc.sync.dma_start(mean_col[:], meanin[:])

            # ---- persistent activations ----
            qk_sb = [main.tile([128, SEQ], bf16, name=f"qk{i}") for i in range(4)]
            v_sb = [main.tile([128, HPC * 128], f32r, name=f"v{t}") for t in range(NST)]
            w_sb = [main.tile([128, SEQ], f32r, name=f"w{h}") for h in range(HPC)]
            outT_sb = [main.tile([128, SEQ], bf16, name=f"outT{h}") for h in range(HPC)]
            wo_sb = [main.tile([128, SEQ], bf16, name=f"wo{i}") for i in range(HPC)]
            d1_all = [main.tile([1, SEQ], f32, name=f"d1a{h}") for h in range(HPC)]

            def bcast(vec_ap, out_ap, n):
                # [1, n] sbuf -> dram -> [128, n] sbuf partition-broadcast
                s = drp.tile([1, n], f32, tag="dbc")
                nc.sync.dma_start(s[:], vec_ap)
                bap = bass.AP(tensor=s.tensor, offset=s.offset,
                              ap=[[0, 128]] + list(s.ap[1:]))
                nc.sync.dma_start(out_ap, bap)


            def ln_chunk(h, c, pe_, pt_, ps_):
                csl = slice(CHUNK * c, CHUNK * (c + 1))
                wsq = pt_.tile([128, CHUNK], f32r, tag="wsq", name="wsq")
                nc.vector.tensor_tensor(wsq[:], w_sb[h][:, csl],
                                        w_sb[h][:, csl], OP.mult)
                s1f = pe_.tile([128, CHUNK], f32, tag="e", name="s1f")
                s2f = pe_.tile([128, CHUNK], f32, tag="e", name="s2f")
                s1 = s1f[0:1, :]
                s2 = s2f[0:1, :]
                nc.tensor.matmul(s1, mean_col[:], w_sb[h][:, csl],
                                 start=True, stop=True)
                nc.tensor.matmul(s2, mean_col[:], wsq[:], start=True, stop=True)
                mu = ps_.tile([1, CHUNK], f32, tag="sm", name="mu")
                nc.vector.tensor_copy(mu[:], s1)
                mu2 = ps_.tile([1, CHUNK], f32, tag="sm", name="mu2")
                nc.scalar.activation(mu2[:], mu[:], AT.Square)
                var = ps_.tile([1, CHUNK], f32, tag="sm", name="var")
                nc.vector.tensor_tensor(var[:], s2, mu2[:], OP.subtract)
                d1sq = ps_.tile([1, CHUNK], f32, tag="sm", name="d1sq")
                nc.scalar.activation(d1sq[:], d1_all[h][:, csl], AT.Square)
                varep = ps_.tile([1, CHUNK], f32, tag="sm", name="varep")
                nc.vector.scalar_tensor_tensor(
                    varep[:], in0=d1sq[:], scalar=LN_EPS, in1=var[:],
                    op0=OP.mult, op1=OP.add)
                # rsqrt via exp(-0.5*ln(x)) — same ACT table set as exp
                lnv = ps_.tile([1, CHUNK], f32, tag="sm", name="lnv")
                nc.scalar.activation(lnv[:], varep[:], AT.Ln)
                rsd = ps_.tile([1, CHUNK], f32, tag="sm", name="rsd")
                nc.scalar.activation(rsd[:], lnv[:], AT.Exp, scale=-0.5)
                mrs = ps_.tile([1, CHUNK], f32, tag="sm", name="mrs")
                nc.vector.tensor_tensor(mrs[:], mu[:], rsd[:], OP.mult)
                rsd_b = pt_.tile([128, CHUNK], f32, tag="rsdb", name="rsd_b")
                bcast(rsd[:], rsd_b[:], CHUNK)
                mrs_b = pt_.tile([128, CHUNK], f32, tag="mrsb", name="mrs_b")
                bcast(mrs[:], mrs_b[:], CHUNK)
                u1 = pt_.tile([128, CHUNK], f32, tag="u1", name="u1")
                nc.vector.tensor_tensor(u1[:], w_sb[h][:, csl], rsd_b[:], OP.mult)
                u2 = pt_.tile([128, CHUNK], f32, tag="u2", name="u2")
                nc.vector.tensor_tensor(u2[:], u1[:], mrs_b[:], OP.subtract)
                nc.vector.tensor_scalar(
                    outT_sb[h][:, csl], u2[:], gam_sb[h][:], bet_sb[h][:],
                    OP.mult, OP.add)

            def proj_chunk(c, py_, pt_):
                for st in range(4 * c, 4 * (c + 1)):
                    ssl = slice(128 * st, 128 * (st + 1))
                    for oc in range(NCHUNK):
                        osl = slice(CHUNK * oc, CHUNK * (oc + 1))
                        yp = py_.tile([128, CHUNK], f32, tag="e", name="yp")
                        for i in range(HPC):
                            nc.tensor.matmul(
                                yp[:], outT_sb[i][:, ssl], wo_sb[i][:, osl],
                                start=(i == 0), stop=(i == HPC - 1))
                        ys = pt_.tile([128, CHUNK], bf16, tag="ys", name="ys")
                        if (st + oc) % 2 == 0:
                            nc.vector.tensor_copy(ys[:], yp[:])
                        else:
                            nc.scalar.copy(ys[:], yp[:])
                        nc.sync.dma_start(y[ssl, osl], ys[:])

            # ================= phase 1: qkv projection =================
            with (
                tc.tile_pool(name="p1w", bufs=1) as p1w,
                tc.tile_pool(name="p1x", bufs=26) as p1x,
                tc.tile_pool(name="p1ps", bufs=4, space="PSUM") as p1ps,
            ):
                wqk_t = [p1w.tile([128, 512], bf16, name=f"wqk{d}") for d in range(NDT)]
                wv_t = [p1w.tile([128, HPC * 128], bf16, name=f"wv{d}") for d in range(NDT)]

                for c in range(NCHUNK):
                    xc = []
                    for d in range(NDT):
                        if c == 0:
                            nc.sync.dma_start(
                                wqk_t[d][:], wqkT[128 * d:128 * (d + 1), :])
                        t = p1x.tile([128, CHUNK], bf16, tag="xc")
                        nc.sync.dma_start(
                            t[:], xT[128 * d:128 * (d + 1), CHUNK * c:CHUNK * (c + 1)])
                        xc.append(t)
                        if c == 0:
                            nc.sync.dma_start(
                                wv_t[d][:], wvT[128 * d:128 * (d + 1), :])
                    if c == 0:
                        load_consts()
                    if c == 1:
                        for i in range(HPC):
                            nc.sync.dma_start(
                                wo_sb[i][:], woT[128 * i:128 * (i + 1), :])
                    # d-outer accumulation: compute starts once the first
                    # d-tile of weights+x has landed
                    qps = [p1ps.tile([128, CHUNK], f32, tag="qkps", name=f"qps{ct}")
                           for ct in range(4)]
                    vps = [p1ps.tile([128, HPC * 128], f32, tag="vps", name=f"vps{ss}")
                           for ss in range(4)]
                    for d in range(NDT):
                        for ct in range(4):
                            nc.tensor.matmul(
                                qps[ct][:], wqk_t[d][:, 128 * ct:128 * (ct + 1)],
                                xc[d][:], start=(d == 0), stop=(d == NDT - 1))
                        for ss in range(4):
                            nc.tensor.matmul(
                                vps[ss][:], xc[d][:, 128 * ss:128 * (ss + 1)],
                                wv_t[d][:], start=(d == 0), stop=(d == NDT - 1))
                    for ct in range(4):
                        nc.vector.tensor_copy(
                            qk_sb[ct][:, CHUNK * c:CHUNK * (c + 1)], qps[ct][:])
                    for ss in range(4):
                        nc.vector.tensor_copy(v_sb[4 * c + ss][:], vps[ss][:])

            # ===== phase 2: differential attention =====
            # A/d psums drain to SBUF immediately at chunk end so
            # single-buffered accumulators release fast; 4 exp-psum slots
            # decouple the score matmuls from ACT.
            with (
                tc.tile_pool(name="p2e", bufs=4, space="PSUM") as p2e,
                tc.tile_pool(name="p2a1", bufs=1, space="PSUM") as p2a1,
                tc.tile_pool(name="p2a2", bufs=1, space="PSUM") as p2a2,
                tc.tile_pool(name="p2d1", bufs=1, space="PSUM") as p2d1,
                tc.tile_pool(name="p2d2", bufs=1, space="PSUM") as p2d2,
                tc.tile_pool(name="p2sb", bufs=10) as p2sb,
                tc.tile_pool(name="p3t", bufs=2) as p3t,
                tc.tile_pool(name="p3s", bufs=4) as p3s,
                tc.tile_pool(name="p3ys", bufs=4) as p3ys,
                tc.tile_pool(name="p2t", bufs=4) as p2t,
                tc.tile_pool(name="p2s", bufs=8) as p2s,
            ):
                for h in range(HPC):
                    qT = qk_sb[h]
                    kT = qk_sb[2 + h]
                    for c in range(NCHUNK):
                        n_sk = 4 * (c + 1)
                        csl = slice(CHUNK * c, CHUNK * (c + 1))
                        a1 = p2a1.tile([128, CHUNK], f32, tag="a1")
                        a2 = p2a2.tile([128, CHUNK], f32, tag="a2")
                        d1 = p2d1.tile([1, CHUNK], f32, tag="d1")
                        d2 = p2d2.tile([1, CHUNK], f32, tag="d2")
                        def scores_part(t):
                            # scores + exp (+mask) for tile t
                            diag = t >= 4 * c
                            f0 = 128 * (t - 4 * c) if diag else 0
                            sl = slice(f0, CHUNK)
                            qsl = slice(CHUNK * c + f0, CHUNK * (c + 1))
                            e1p = p2e.tile([128, CHUNK], f32, tag="e", name="e1p")
                            e2p = p2e.tile([128, CHUNK], f32, tag="e", name="e2p")
                            nc.tensor.matmul(
                                e1p[:, sl], kT[0:64, 128 * t:128 * (t + 1)],
                                qT[0:64, qsl], start=True, stop=True)
                            nc.tensor.matmul(
                                e2p[:, sl], kT[64:128, 128 * t:128 * (t + 1)],
                                qT[64:128, qsl], start=True, stop=True)
                            e1 = p2sb.tile([128, CHUNK], f32r, tag="e1", name="e1")
                            e2 = p2sb.tile([128, CHUNK], f32r, tag="e2", name="e2")
                            nc.scalar.activation(e1[:, sl], e1p[:, sl], AT.Exp)
                            nc.scalar.activation(e2[:, sl], e2p[:, sl], AT.Exp)
                            if diag:
                                dsl = slice(f0, f0 + 128)
                                nc.vector.tensor_tensor(
                                    e1[:, dsl], e1[:, dsl], tri_sb[:], OP.mult)
                                nc.vector.tensor_tensor(
                                    e2[:, dsl], e2[:, dsl], tri_sb[:], OP.mult)
                            return e1, e2, sl

                        # 1-tile emission skew: scores(t+1) enters the PE queue
                        # before PV(t), so the in-order PE never stalls on exp(t)
                        prev = scores_part(0)
                        for t in range(n_sk):
                            nxt = scores_part(t + 1) if t + 1 < n_sk else None
                            e1, e2, sl = prev
                            first, last = (t == 0), (t == n_sk - 1)
                            vt = v_sb[t][:, 128 * h:128 * (h + 1)]
                            nc.tensor.matmul(a1[:, sl], vt, e1[:, sl],
                                             start=first, stop=last)
                            nc.tensor.matmul(a2[:, sl], vt, e2[:, sl],
                                             start=first, stop=last)
                            nc.tensor.matmul(d1[:, sl], ones_col[:], e1[:, sl],
                                             start=first, stop=last)
                            nc.tensor.matmul(d2[:, sl], ones_col[:], e2[:, sl],
                                             start=first, stop=last)
                            prev = nxt
                        # drain psums to SBUF fast, then combine from SBUF:
                        # w' = A1u - (d1/d2)*lam*A2u   (LN scale-invariance)
                        a1s = p2t.tile([128, CHUNK], f32, tag="a1s")
                        a2s = p2t.tile([128, CHUNK], f32, tag="a2s")
                        nc.vector.tensor_copy(a1s[:], a1[:])
                        nc.vector.tensor_copy(a2s[:], a2[:])
                        nc.vector.tensor_copy(d1_all[h][:, csl], d1[:])
                        d2s = p2s.tile([1, CHUNK], f32, tag="rd")
                        nc.vector.tensor_copy(d2s[:], d2[:])
                        rd2 = p2s.tile([1, CHUNK], f32, tag="rd")
                        nc.vector.reciprocal(rd2[:], d2s[:])
                        r = p2s.tile([1, CHUNK], f32, tag="rd")
                        nc.vector.tensor_tensor(r[:], d1_all[h][:, csl], rd2[:],
                                                OP.mult)
                        rb = p2t.tile([128, CHUNK], f32, tag="rb")
                        bcast(r[:], rb[:], CHUNK)
                        ta2 = p2t.tile([128, CHUNK], f32, tag="ta2")
                        nc.vector.tensor_tensor(ta2[:], a2s[:], rb[:], OP.mult)
                        nc.vector.scalar_tensor_tensor(
                            w_sb[h][:, csl],
                            in0=ta2[:], scalar=lam_sb[:], in1=a1s[:],
                            op0=OP.mult, op1=OP.add)
                # ---- tail: LN + projection, psum via the shared e slots
                # (inside the phase-2 scope: no pool-boundary address-reuse
                # dependency on the last exp slots) ----
                for c in range(NCHUNK):
                    ln_chunk(0, c, p2e, p3t, p3s)
                    ln_chunk(1, c, p2e, p3t, p3s)
                    proj_chunk(c, p2e, p3ys)

    from concourse import mybir as _mb
    _fix_sync_limits(nc, _mb, bass_rust)
    return nc


_NC_CACHE = {}


def _get_nc():
    if "nc" not in _NC_CACHE:
        _NC_CACHE["nc"] = _build_nc()
    return _NC_CACHE["nc"]


def kernel(x, W_qkv, W_o, lambda_q1, lambda_k1, lambda_q2, lambda_k2,
           gn_gamma, gn_beta):
    import os
    _install_ntff_shim()
    from concourse.bass_utils import run_bass_kernel_spmd

    x = np.asarray(x, np.float32)
    W_qkv = np.asarray(W_qkv, np.float32)
    W_o = np.asarray(W_o, np.float32)
    lambda_q1 = np.asarray(lambda_q1, np.float32)
    lambda_k1 = np.asarray(lambda_k1, np.float32)
    lambda_q2 = np.asarray(lambda_q2, np.float32)
    lambda_k2 = np.asarray(lambda_k2, np.float32)
    gn_gamma = np.asarray(gn_gamma, np.float32)
    gn_beta = np.asarray(gn_beta, np.float32)

    lambda_init = np.float32(0.8 - 0.6 * np.exp(-0.3 * LAYER_IDX))
    lam = (np.exp(lambda_q1 * lambda_k1) - np.exp(lambda_q2 * lambda_k2)
           + lambda_init).astype(np.float32)
    one_m_li = np.float32(1.0 - lambda_init)
    scale = np.float32(HEAD_DIM ** -0.5)

    import ml_dtypes
    xT = np.ascontiguousarray(x[0].T).astype(ml_dtypes.bfloat16)
    W3 = W_qkv.reshape(3, N_HEADS, 128, D_MODEL)
    tri = (np.arange(512)[None, :128] >= np.arange(128)[:, None])
    trimask = np.ascontiguousarray(tri[:, :128]).astype(np.float32)
    onesin = np.ones((128, 128), np.float32)
    meanin = np.full((128, 1), 1.0 / 128, np.float32)

    in_maps = []
    for i in range(N_CORES):
        hs = [HPC * i + k for k in range(HPC)]
        wq = np.concatenate([W3[0, h] * scale for h in hs], 0)   # [256, D]
        wk = np.concatenate([W3[1, h] for h in hs], 0)           # [256, D]
        wv = np.concatenate([W3[2, h] for h in hs], 0)           # [256, D]
        wqkT = np.ascontiguousarray(np.concatenate([wq, wk], 0).T).astype(ml_dtypes.bfloat16)
        wvT = np.ascontiguousarray(wv.T).astype(ml_dtypes.bfloat16)
        woT = np.ascontiguousarray(W_o[:, 128 * hs[0]:128 * (hs[-1] + 1)].T).astype(ml_dtypes.bfloat16)
        in_maps.append({
            "xT": xT,
            "wqkT": wqkT,
            "wvT": wvT,
            "woT": woT,
            "lamneg": np.ascontiguousarray(-lam),
            "gamma": np.ascontiguousarray(gn_gamma[hs] * one_m_li),
            "beta": np.ascontiguousarray(gn_beta[hs] * one_m_li),
            "trimask": trimask,
            "onesin": onesin,
            "meanin": meanin,
        })

    nc = _get_nc()
    trace = bool(int(os.environ.get("KERNEL_TRACE", "0")))
    res = run_bass_kernel_spmd(nc, in_maps, core_ids=list(range(N_CORES)),
                               trace=trace)
    if trace:
        _NC_CACHE["last_result"] = res
    y = np.zeros((SEQ, D_MODEL), np.float32)
    for r in res.results:
        y += np.asarray(r["y"], np.float32)
    return y[None]



# revision 16
# speedup vs baseline: 1.2601x; 1.2601x over previous
"""DifferentialAttention on 8 TRN2 NeuronCores.

Sharding: tensor-parallel over heads (2 heads per core), host sums the
8 partial output projections (not counted in HW exec time).

Restructured pipeline (v2):
- qkv projection with d-inner accumulation sharing top-level PSUM pools,
  fully interleaved with attention chunks (no phase barrier).
- Attention uses a TRANSPOSED PV: out[q, ch] with the softmax
  denominator as a free 129th "ones" column of V — the two ones-row
  matmuls per key tile of v1 are gone (-29us PE).
- lambda folded into a second copy of V (v' = v * -lam) so the
  differential combine is ONE scalar_tensor_tensor with a per-partition
  scalar (d2/d1); LN runs over the free dim with per-partition scalars;
  no DMA/matmul broadcasts anywhere.
- gamma*(1-lam_init) folded into W_o rows on host; beta handled as a
  host-side rank-1 bias.
- Per-head LN output transposed back via PE transpose matmuls; output
  projection interleaved between attention chunks.
"""

import numpy as np

HEAD_DIM = 64
N_HEADS = 16
D_MODEL = 2048
SEQ = 2048
LAYER_IDX = 12
LN_EPS = 1e-5
N_CORES = 8
HPC = N_HEADS // N_CORES          # heads per core = 2
CHUNK = 512                       # query chunk width
NCHUNK = SEQ // CHUNK             # 4
NDT = D_MODEL // 128              # 16 d-tiles
NST = SEQ // 128                  # 16 s-tiles

_SYNC_CNT = [0]


def _patch_tile_drain(tile_mod, bass_rust):
    """The walrus build in this container encodes at most one sem wait per
    instruction; TileContext's exit drain carries one wait per producer
    proc. Split the extras onto single-wait NOPs."""
    from concourse.vector_clock import ScopedClock

    def patched(self, tick_clock, wait_clock):
        nc = self.nc
        drain_inst = nc.sync.drain()
        wait_clock.add_sem_waits(
            drain_inst.ins, ScopedClock({None: tick_clock.global_clock})
        )
        si = drain_inst.ins.sync_info
        waits = list(si.on_wait or [])
        if len(waits) > 1:
            si.on_wait = [waits[0]]
            for w in waits[1:]:
                nop = nc.sync.nop()
                nop.ins.sync_info = bass_rust.SyncInfo(on_wait=[w], on_update=[])
        nc.all_engine_barrier()
        popped = nc._tile_sem_poison_stack.pop()
        assert popped is self._sem_poison
        nc.clear_and_free_semaphores(list(self.sems.allocated().values()))
        nc.all_engine_barrier()

    tile_mod.TileContext._drain_and_barrier = patched


def _fix_sync_limits(nc, mybir, bass_rust):
    """Split multi-wait / multi-update instructions into single-wait NOP
    chains on the same engine queue (walrus single-sync-slot limit)."""

    def nop(engine, wait=None, update=None):
        _SYNC_CNT[0] += 1
        n = mybir.InstNoOp(name=f"syncsplit-{_SYNC_CNT[0]}", ins=[], outs=[])
        n.engine = engine
        n.sync_info = bass_rust.SyncInfo(
            on_wait=[wait] if wait is not None else [],
            on_update=[update] if update is not None else [],
        )
        return n

    for f in nc.m.functions:
        for b in f.blocks:
            out = []
            for inst in b.instructions:
                si = inst.sync_info
                post = []
                if si is not None:
                    waits = list(si.on_wait or [])
                    if len(waits) > 1:
                        for w in waits[:-1]:
                            out.append(nop(inst.engine, wait=w))
                        si.on_wait = [waits[-1]]
                    ups = list(si.on_update or [])
                    if len(ups) > 1:
                        si.on_update = [ups[0]]
                        for u in ups[1:]:
                            post.append(nop(inst.engine, update=u))
                out.append(inst)
                out.extend(post)
            b.instructions = out


def _install_ntff_shim():
    """Register the axon NTFF profile hook (used only when tracing)."""
    import sys, types
    if "antenv.axon_hooks" in sys.modules:
        return
    try:
        mod = types.ModuleType("antenv.axon_hooks")
        mod._hook = None
        mod.set_axon_ntff_profile_hook = lambda h: setattr(mod, "_hook", h)
        mod.get_axon_ntff_profile_hook = lambda: mod._hook
        sys.modules["antenv.axon_hooks"] = mod
        import antenv
        antenv.axon_hooks = mod
        from trn_agent_boot.trn_boot import _ntff_profile_via_ctypes
        mod.set_axon_ntff_profile_hook(
            _ntff_profile_via_ctypes("/opt/axon/libaxon_pjrt.so")
        )
    except Exception:
        pass


def _build_nc():
    import os
    GPS_TT = bool(int(os.environ.get("GPS_TT", "1")))
    GPS_MS = bool(int(os.environ.get("GPS_MS", "1")))
    STAGE = int(os.environ.get("STAGE", "4"))
    import bass_rust
    import concourse.bass as bass
    import concourse.tile as tile
    import concourse.tile_sem_assignment as _tsa
    from concourse import mybir

    _patch_tile_drain(tile, bass_rust)
    # The Pool-engine proc sem plus 8 HWDGE sems overflows the sem range
    # this walrus build can encode in sem_clear; 7 DMA queues suffice.
    _tsa.NUM_HWDGE_SEMS = 7

    f32 = mybir.dt.float32
    bf16 = mybir.dt.bfloat16
    AT = mybir.ActivationFunctionType
    OP = mybir.AluOpType
    AX = mybir.AxisListType

    nc = bass.Bass()

    xT = nc.dram_tensor("xT", [D_MODEL, SEQ], bf16, kind="ExternalInput")
    wqkT = nc.dram_tensor("wqkT", [D_MODEL, 4 * 128], bf16, kind="ExternalInput")
    wvT = nc.dram_tensor("wvT", [D_MODEL, HPC * 128], bf16, kind="ExternalInput")
    woT = nc.dram_tensor("woT", [HPC * 128, D_MODEL], bf16, kind="ExternalInput")
    lamnegbc = nc.dram_tensor("lamnegbc", [128, HPC * 128], f32, kind="ExternalInput")
    tri2 = nc.dram_tensor("tri2", [128, 256], bf16, kind="ExternalInput")
    ident = nc.dram_tensor("ident", [128, 128], bf16, kind="ExternalInput")
    y = nc.dram_tensor("y", [SEQ, D_MODEL], bf16, kind="ExternalOutput")

    SQEPS = float(np.sqrt(LN_EPS))

    with tile.TileContext(nc) as tc:
        import contextlib
        with contextlib.ExitStack() as ctx:
            consts = ctx.enter_context(tc.tile_pool(name="consts", bufs=1))
            main = ctx.enter_context(tc.tile_pool(name="main", bufs=1))
            p1w = ctx.enter_context(tc.tile_pool(name="p1w", bufs=1))
            p1x = ctx.enter_context(tc.tile_pool(name="p1x", bufs=26))
            pe12 = ctx.enter_context(tc.tile_pool(name="pe12", bufs=18))
            pw = ctx.enter_context(tc.tile_pool(name="pw", bufs=2))
            pot = ctx.enter_context(tc.tile_pool(name="pot", bufs=3))
            ppo = ctx.enter_context(tc.tile_pool(name="ppo", bufs=4))
            pyr = ctx.enter_context(tc.tile_pool(name="pyr", bufs=2))
            psm = ctx.enter_context(tc.tile_pool(name="psm", bufs=8))
            # PSUM: 4 + 3 + 1 = 8 banks
            pe = ctx.enter_context(tc.tile_pool(name="pe", bufs=4, space="PSUM"))
            pa = ctx.enter_context(tc.tile_pool(name="pa", bufs=3, space="PSUM"))
            ptr = ctx.enter_context(tc.tile_pool(name="ptr", bufs=1, space="PSUM"))

            # ---- constants ----
            lam_bc = consts.tile([128, HPC * 128], f32)
            tri_sb = consts.tile([128, 2, 128], bf16)
            id_sb = consts.tile([128, 128], bf16)

            def load_consts():
                nc.sync.dma_start(lam_bc[:], lamnegbc[:])
                nc.sync.dma_start(tri_sb[:], tri2[:])
                nc.sync.dma_start(id_sb[:], ident[:])

            # ---- persistent activations ----
            qk_sb = [main.tile([128, SEQ], bf16, name=f"qk{i}") for i in range(4)]
            # v tile layout per 128-key block: [head][v(128) | 1 | v'(128) | 1]
            v_sb = [main.tile([128, HPC, 258], bf16, name=f"v{t}")
                    for t in range(NST)]
            wo_sb = [main.tile([128, SEQ], bf16, name=f"wo{i}") for i in range(HPC)]
            wqk_t = [p1w.tile([128, 512], bf16, name=f"wqk{d}") for d in range(NDT)]
            wv_t = [p1w.tile([128, HPC * 128], bf16, name=f"wv{d}") for d in range(NDT)]

            # =================== phase-1 chunk ===================
            def ph1(c):
                csl = slice(CHUNK * c, CHUNK * (c + 1))
                xc = []
                for d in range(NDT):
                    if c == 0:
                        nc.sync.dma_start(
                            wqk_t[d][:], wqkT[128 * d:128 * (d + 1), :])
                    t = p1x.tile([128, CHUNK], bf16, tag="xc")
                    nc.sync.dma_start(t[:], xT[128 * d:128 * (d + 1), csl])
                    xc.append(t)
                    if c == 0:
                        nc.sync.dma_start(
                            wv_t[d][:], wvT[128 * d:128 * (d + 1), :])
                if c == 0:
                    load_consts()
                    # ones columns of every v tile (cols 128, 257 per head);
                    # keep gpsimd APs 2D (Q7 SW ops)
                    _mse = nc.gpsimd if GPS_MS else nc.vector
                    for t in range(NST):
                        for hh in range(HPC):
                            _mse.memset(v_sb[t][:, hh, 128:129], 1.0)
                            _mse.memset(v_sb[t][:, hh, 257:258], 1.0)
                if c == 1:
                    for i in range(HPC):
                        nc.sync.dma_start(
                            wo_sb[i][:], woT[128 * i:128 * (i + 1), :])
                # q/k: d-inner accumulation, one live accumulator per ct
                for ct in range(4):
                    qps = pe.tile([128, CHUNK], f32, tag="e", name=f"qps{ct}")
                    for d in range(NDT):
                        nc.tensor.matmul(
                            qps[:], wqk_t[d][:, 128 * ct:128 * (ct + 1)],
                            xc[d][:], start=(d == 0), stop=(d == NDT - 1))
                    nc.vector.tensor_copy(qk_sb[ct][:, csl], qps[:])
                # v: per s-block, both heads wide
                for ss in range(4):
                    t = 4 * c + ss
                    vps = pa.tile([128, 258], f32, tag="acc", name=f"vps{ss}")
                    for d in range(NDT):
                        nc.tensor.matmul(
                            vps[:, 0:256], xc[d][:, 128 * ss:128 * (ss + 1)],
                            wv_t[d][:], start=(d == 0), stop=(d == NDT - 1))
                    for hh in range(HPC):
                        hsl = slice(128 * hh, 128 * (hh + 1))
                        nc.vector.tensor_copy(
                            v_sb[t][:, hh, 0:128], vps[:, hsl])
                        # gpsimd cannot read PSUM: derive v' from the sbuf copy
                        (nc.gpsimd if GPS_TT else nc.vector).tensor_tensor(
                            v_sb[t][:, hh, 129:257], v_sb[t][:, hh, 0:128],
                            lam_bc[:, hsl], OP.mult)

            # =================== attention block ===================
            def attn(h, c):
                qT = qk_sb[h]
                kT = qk_sb[2 + h]
                n_sk = 4 * (c + 1)
                e12s = [None] * n_sk

                def scores_t(t):
                    diag = t >= 4 * c
                    f0 = 128 * (t - 4 * c) if diag else 0
                    sl = slice(f0, CHUNK)
                    qsl = slice(CHUNK * c + f0, CHUNK * (c + 1))
                    e1p = pe.tile([128, CHUNK], f32, tag="e", name="e1p")
                    e2p = pe.tile([128, CHUNK], f32, tag="e", name="e2p")
                    nc.tensor.matmul(
                        e1p[:, sl], kT[0:64, 128 * t:128 * (t + 1)],
                        qT[0:64, qsl], start=True, stop=True)
                    nc.tensor.matmul(
                        e2p[:, sl], kT[64:128, 128 * t:128 * (t + 1)],
                        qT[64:128, qsl], start=True, stop=True)
                    et = pe12.tile([128, 2, CHUNK], bf16, tag="e12")
                    nc.scalar.activation(et[:, 0, sl], e1p[:, sl], AT.Exp)
                    nc.scalar.activation(et[:, 1, sl], e2p[:, sl], AT.Exp)
                    if diag:
                        dsl = slice(f0, f0 + 128)
                        nc.vector.tensor_tensor(
                            et[:, :, dsl], et[:, :, dsl], tri_sb[:], OP.mult)
                    e12s[t] = et

                # w in [:, 0:4, :], w^2 in [:, 4:8, :] -> one batched reduce
                w_t = pw.tile([128, 8, 128], f32, tag="w")
                s18 = psm.tile([128, 8], f32, tag="s18")
                epsd2 = psm.tile([128, 4], f32, tag="ed")

                for t in range(4 * c + 1):
                    scores_t(t)

                for j in range(4):
                    if 4 * c + j + 1 < n_sk:
                        scores_t(4 * c + j + 1)
                    nt = 4 * c + j + 1
                    a1t = pa.tile([128, 258], f32, tag="acc", name="a1")
                    a2t = pa.tile([128, 258], f32, tag="acc", name="a2")
                    jsl = slice(128 * j, 128 * (j + 1))
                    for t in range(nt):
                        nc.tensor.matmul(
                            a1t[:, 0:129], e12s[t][:, 0, jsl],
                            v_sb[t][:, h, 0:129],
                            start=(t == 0), stop=(t == nt - 1))
                        nc.tensor.matmul(
                            a2t[:, 0:129], e12s[t][:, 1, jsl],
                            v_sb[t][:, h, 129:258],
                            start=(t == 0), stop=(t == nt - 1))
                    # w = (d2/d1)*a1 + a2'   (= d2 * w_true, LN-scale-invariant)
                    rd1 = psm.tile([128, 1], f32, tag="rd1")
                    nc.vector.reciprocal(rd1[:], a1t[:, 128:129])
                    scol = psm.tile([128, 1], f32, tag="scol")
                    nc.vector.tensor_tensor(
                        scol[:], a2t[:, 128:129], rd1[:], OP.mult)
                    # DVE reads at most one PSUM operand per instruction:
                    # (d2/d1)*a1 -> sbuf, then + a2' from the other psum
                    sa1 = pw.tile([128, 128], f32, tag="sa1")
                    nc.vector.tensor_scalar_mul(sa1[:], a1t[:, 0:128], scol[:])
                    nc.vector.tensor_tensor(
                        w_t[:, j], sa1[:], a2t[:, 0:128], OP.add)
                    nc.scalar.activation(
                        epsd2[:, j:j + 1], a2t[:, 128:129], AT.Square,
                        scale=SQEPS)
                    nc.scalar.activation(w_t[:, 4 + j], w_t[:, j], AT.Square)

                # ---- LN stats (free-dim, per-partition) ----
                nc.vector.tensor_reduce(s18[:], w_t[:], axis=AX.X, op=OP.add)
                s1c = s18[:, 0:4]
                s2c = s18[:, 4:8]
                t0 = psm.tile([128, 4], f32, tag="t0")
                nc.vector.scalar_tensor_tensor(
                    t0[:], in0=s1c, scalar=1.0 / 128, in1=s1c,
                    op0=OP.mult, op1=OP.mult)
                t1 = psm.tile([128, 4], f32, tag="t1")
                nc.vector.tensor_tensor(t1[:], s2c, t0[:], OP.subtract)
                varep = psm.tile([128, 4], f32, tag="ve")
                nc.vector.scalar_tensor_tensor(
                    varep[:], in0=t1[:], scalar=1.0 / 128, in1=epsd2[:],
                    op0=OP.mult, op1=OP.add)
                lnv = psm.tile([128, 4], f32, tag="lnv")
                nc.scalar.activation(lnv[:], varep[:], AT.Ln)
                rstd = psm.tile([128, 4], f32, tag="rstd")
                nc.scalar.activation(rstd[:], lnv[:], AT.Exp, scale=-0.5)
                nmr = psm.tile([128, 4], f32, tag="nmr")
                nc.vector.scalar_tensor_tensor(
                    nmr[:], in0=s1c, scalar=-1.0 / 128, in1=rstd[:],
                    op0=OP.mult, op1=OP.mult)
                outT_t = pot.tile([128, 4, 128], bf16, tag="outT")
                for j in range(4):
                    nc.scalar.activation(
                        outT_t[:, j], w_t[:, j], AT.Identity,
                        scale=rstd[:, j:j + 1], bias=nmr[:, j:j + 1])
                return outT_t

            # =================== transpose + store ===================
            def tr(outT_t):
                trp = ptr.tile([128, 4, 128], bf16, tag="tr")
                for j in range(4):
                    nc.tensor.matmul(
                        trp[:, j], outT_t[:, j],
                        id_sb[:], is_transpose=True)
                po = ppo.tile([128, 4, 128], bf16, tag="po")
                nc.vector.tensor_copy(po[:], trp[:])
                return po

            # =================== projection chunk ===================
            def proj(c, po_pair):
                for st_l in range(4):
                    st = 4 * c + st_l
                    yr = pyr.tile([128, SEQ], bf16, tag="yr")
                    for oc in range(4):
                        osl = slice(512 * oc, 512 * (oc + 1))
                        yp = pe.tile([128, 512], f32, tag="e", name="yp")
                        for i in range(HPC):
                            nc.tensor.matmul(
                                yp[:], po_pair[i][:, st_l], wo_sb[i][:, osl],
                                start=(i == 0), stop=(i == HPC - 1))
                        if oc % 2 == 0:
                            nc.vector.tensor_copy(yr[:, osl], yp[:])
                        else:
                            nc.scalar.copy(yr[:, osl], yp[:])
                    nc.sync.dma_start(y[128 * st:128 * (st + 1), :], yr[:])

            # =================== schedule ===================
            if STAGE >= 1:
                ph1(0)
                ph1(1)
            if STAGE >= 2:
                o00 = attn(0, 0)
            if STAGE >= 1:
                ph1(2)
            if STAGE >= 3:
                po00 = tr(o00)
            if STAGE >= 2:
                o10 = attn(1, 0)
            if STAGE >= 1:
                ph1(3)
            if STAGE >= 3:
                po10 = tr(o10)
            if STAGE >= 2:
                o01 = attn(0, 1)
            if STAGE >= 4:
                proj(0, [po00, po10])
            if STAGE >= 3:
                po01 = tr(o01)
            if STAGE >= 2:
                o11 = attn(1, 1)
            if STAGE >= 3:
                po11 = tr(o11)
            if STAGE >= 2:
                o02 = attn(0, 2)
            if STAGE >= 4:
                proj(1, [po01, po11])
            if STAGE >= 3:
                po02 = tr(o02)
            if STAGE >= 2:
                o12 = attn(1, 2)
            if STAGE >= 3:
                po12 = tr(o12)
            if STAGE >= 2:
                o03 = attn(0, 3)
            if STAGE >= 4:
                proj(2, [po02, po12])
            if STAGE >= 3:
                po03 = tr(o03)
            if STAGE >= 2:
                o13 = attn(1, 3)
            if STAGE >= 3:
                po13 = tr(o13)
            if STAGE >= 4:
                proj(3, [po03, po13])
            if STAGE < 4:
                # dummy y write so the output tensor has a writer
                yr = pyr.tile([128, SEQ], bf16, tag="yr")
                nc.vector.tensor_copy(yr[:, 0:SEQ], qk_sb[0][:, 0:SEQ])
                for st in range(NST):
                    nc.sync.dma_start(y[128 * st:128 * (st + 1), :], yr[:])

    from concourse import mybir as _mb
    _fix_sync_limits(nc, _mb, bass_rust)
    return nc


_NC_CACHE = {}


def _get_nc():
    if "nc" not in _NC_CACHE:
        _NC_CACHE["nc"] = _build_nc()
    return _NC_CACHE["nc"]


def kernel(x, W_qkv, W_o, lambda_q1, lambda_k1, lambda_q2, lambda_k2,
           gn_gamma, gn_beta):
    import os
    _install_ntff_shim()
    from concourse.bass_utils import run_bass_kernel_spmd

    x = np.asarray(x, np.float32)
    W_qkv = np.asarray(W_qkv, np.float32)
    W_o = np.asarray(W_o, np.float32)
    lambda_q1 = np.asarray(lambda_q1, np.float32)
    lambda_k1 = np.asarray(lambda_k1, np.float32)
    lambda_q2 = np.asarray(lambda_q2, np.float32)
    gn_gamma = np.asarray(gn_gamma, np.float32)
    gn_beta = np.asarray(gn_beta, np.float32)
    lambda_k2 = np.asarray(lambda_k2, np.float32)

    lambda_init = np.float32(0.8 - 0.6 * np.exp(-0.3 * LAYER_IDX))
    lam = (np.exp(lambda_q1 * lambda_k1) - np.exp(lambda_q2 * lambda_k2)
           + lambda_init).astype(np.float32)
    one_m_li = np.float32(1.0 - lambda_init)
    scale = np.float32(HEAD_DIM ** -0.5)

    import ml_dtypes
    xT = np.ascontiguousarray(x[0].T).astype(ml_dtypes.bfloat16)
    W3 = W_qkv.reshape(3, N_HEADS, 128, D_MODEL)
    tri = (np.arange(128)[None, :] >= np.arange(128)[:, None])  # [k, q]: k<=q
    tri2 = np.ascontiguousarray(
        np.concatenate([tri, tri], axis=1)).astype(ml_dtypes.bfloat16)
    ident = np.eye(128, dtype=np.float32).astype(ml_dtypes.bfloat16)

    in_maps = []
    for i in range(N_CORES):
        hs = [HPC * i + k for k in range(HPC)]
        wq = np.concatenate([W3[0, h] * scale for h in hs], 0)   # [256, D]
        wk = np.concatenate([W3[1, h] for h in hs], 0)           # [256, D]
        wv = np.concatenate([W3[2, h] for h in hs], 0)           # [256, D]
        wqkT_h = np.ascontiguousarray(
            np.concatenate([wq, wk], 0).T).astype(ml_dtypes.bfloat16)
        wvT_h = np.ascontiguousarray(wv.T).astype(ml_dtypes.bfloat16)
        # gamma*(1-lambda_init) folded into W_o rows
        gfold = (gn_gamma[hs] * one_m_li).reshape(-1)            # [256]
        wo_cols = W_o[:, 128 * hs[0]:128 * (hs[-1] + 1)]         # [D, 256]
        woT_h = np.ascontiguousarray(
            (wo_cols * gfold[None, :]).T).astype(ml_dtypes.bfloat16)
        # -lam per value channel, broadcast to 128 partitions
        lamneg_bc = np.ascontiguousarray(
            np.broadcast_to(-lam[None, :], (128, 2 * HEAD_DIM)))
        lamneg_bc = np.concatenate([lamneg_bc] * HPC, axis=1).astype(np.float32)
        in_maps.append({
            "xT": xT,
            "wqkT": wqkT_h,
            "wvT": wvT_h,
            "woT": woT_h,
            "lamnegbc": np.ascontiguousarray(lamneg_bc),
            "tri2": tri2,
            "ident": ident,
        })

    nc = _get_nc()
    trace = bool(int(os.environ.get("KERNEL_TRACE", "0")))
    res = run_bass_kernel_spmd(nc, in_maps, core_ids=list(range(N_CORES)),
                               trace=trace)
    if trace:
        _NC_CACHE["last_result"] = res
    yacc = np.zeros((SEQ, D_MODEL), np.float32)
    for r in res.results:
        yacc += np.asarray(r["y"], np.float32)
    # host-side rank-1 bias: sum_h W_o[:, h-block] @ (beta_h * (1-lam_init))
    bias = W_o @ (gn_beta.reshape(-1) * one_m_li)
    yacc += bias[None, :]
    return yacc[None]
